# revision 10
# baseline (speedup 1.0000x reference)
"""2-layer multi-head GAT on 8 Trainium2 NeuronCores.

Sharding: nodes partitioned across 8 cores by dst ownership (6250 real nodes
per core, padded to 6272 = 49x128). Edges live on their dst's core, sorted by
dst into 128-dst blocks. Per layer:
  1. per-core GEMM  feat|el|er = h @ [W | W@Al | W@Ar]  (fp16 in, fp32 psum)
     -> local table shard [6272, 384] fp16 rows (256 feat + 4 el), er kept
     in a separate local [6272, 128] fp16 table (256B gather rows)
  2. ONE AllGather publishes all shards -> [50176, 384] (core-major order;
     rows < 25088 indexed via table-half A, rest via half B so dma_gather
     int16 indices stay in range)
  3. per 2-block group: dma_gather of src rows (768B) + er rows (256B),
     attention e-chain (DVE/ACT), selection-matrix aggregation matmuls
     accumulated per dst-block in PSUM (fp16 operands, exact 0/1 lhsT)
  4. flush per block: divide by softmax denominators, ELU (fp16), transpose
     for the next GEMM / final output
"""
import sys
sys.path.insert(0, '/opt/trn_rl_repo')
import numpy as np

N_NODES = 50000
N_EDGES = 800000
IN_DIM = 256
HID = 64
HEADS = 4
NEG_SLOPE = 0.2
N_CORES = 8
NPC = N_NODES // N_CORES          # 6250 real nodes per core
P = 128
NB = 49                            # dst blocks per core
NPAD = NB * P                      # 6272 padded nodes per core
HALF = 4 * NPAD                    # 25088: first table half (cores 0-3)
RTOT = N_CORES * NPAD              # 50176 gathered table rows
ES = 384                           # table row elems fp16 (768B)
ERES = 128                         # er table row elems fp16 (256B)
CG = 260                           # feat + denom columns in agg matmul
GB = 2                             # dst blocks per gather group
PAD_LDST = 999.0


def _wrap_idx(idx_list):
    """[n] int -> [128, n//16] int16 wrapped-in-16 layout, replicated."""
    n = len(idx_list)
    assert n % 16 == 0
    arr = np.asarray(idx_list, np.int16).reshape(n // 16, 16)  # [s, q]
    w16 = arr.T                                                # [16, s]
    return np.tile(w16, (8, 1))                                # [128, s]


def _prep(x, src, dst, W1, al1, ar1, W2, al2, ar2, kdt=16):
    src = np.asarray(src).astype(np.int64)
    dst = np.asarray(dst).astype(np.int64)
    x = np.asarray(x, np.float32)

    own = (src // NPC).astype(np.int32)
    loc = (src % NPC).astype(np.int32)
    in_a = own < 4
    rowA = own * NPAD + loc                   # valid where in_a  (< 25088)
    rowB = (own - 4) * NPAD + loc             # valid where ~in_a (< 25088)

    core_of = (dst // NPC).astype(np.int32)
    ld_all = (dst % NPC).astype(np.int32)
    blk_all = ld_all // P
    lin_all = ld_all % P

    # per (core, block): lists of A-edges and B-edges
    eA = [[[] for _ in range(NB)] for _ in range(N_CORES)]
    eB = [[[] for _ in range(NB)] for _ in range(N_CORES)]
    order = np.lexsort((src, dst))
    for e in order:
        c = core_of[e]
        b = blk_all[e]
        (eA if in_a[e] else eB)[c][b].append(e)

    T_A = [max(len(eA[c][b]) for c in range(N_CORES)) for b in range(NB)]
    T_B = [max(len(eB[c][b]) for c in range(N_CORES)) for b in range(NB)]
    T_A = [-(-n // P) for n in T_A]
    T_B = [-(-n // P) for n in T_B]

    # groups of GB consecutive blocks
    groups = [list(range(g, min(g + GB, NB))) for g in range(0, NB, GB)]
    # per group: tile layout [A(b0) A(b1) ... B(b0) B(b1) ...]
    ginfo = []
    for blks in groups:
        ginfo.append({
            "blks": blks,
            "tA": [T_A[b] for b in blks],
            "tB": [T_B[b] for b in blks],
        })
    plan = {"ginfo": ginfo, "T_A": T_A, "T_B": T_B}

    # attention projection: [256, 4] per layer with per-head blocks
    def aext(al, ar):
        Al = np.zeros((IN_DIM, HEADS), np.float64)
        Ar = np.zeros((IN_DIM, HEADS), np.float64)
        for h in range(HEADS):
            Al[h * HID:(h + 1) * HID, h] = np.asarray(al, np.float64)[h]
            Ar[h * HID:(h + 1) * HID, h] = np.asarray(ar, np.float64)[h]
        return Al, Ar

    Al1, Ar1 = aext(al1, ar1)
    Al2, Ar2 = aext(al2, ar2)

    def wext(W, Al, Ar):
        W = np.asarray(W, np.float64)
        m = np.concatenate([W, W @ Al, W @ Ar], axis=1)  # [256, 264]
        out = np.zeros((P, 2 * 264), np.float16)
        for g in range(2):
            out[:, g * 264:(g + 1) * 264] = m[g * P:(g + 1) * P].astype(np.float16)
        return out

    W1k = wext(W1, Al1, Ar1)
    W2k = wext(W2, Al2, Ar2)
    iota = np.tile(np.arange(P, dtype=np.float16), (P, 1))
    ident = np.eye(P, dtype=np.float16)

    in_maps = []
    for c in range(N_CORES):
        xl = np.zeros((NPAD, IN_DIM), np.float32)
        xl[:NPC] = x[c * NPC:(c + 1) * NPC]
        # block-interleaved transpose: xT2[p, b*256 + g*128 + n] = xl[b*128+n, g*128+p]
        xT2 = np.ascontiguousarray(
            xl.reshape(NB, P, 2, P).transpose(3, 0, 2, 1).reshape(P, 2 * NPAD)
        ).astype(np.float16)

        idxA_cols = []
        idxB_cols = []
        idxL_cols = []
        ldst_cols = []
        for gi in ginfo:
            ia, ib, il_a, il_b, lv_a, lv_b = [], [], [], [], [], []
            for k, b in enumerate(gi["blks"]):
                ea, eb = eA[c][b], eB[c][b]
                na, nb_ = gi["tA"][k] * P, gi["tB"][k] * P
                ia += [int(rowA[e]) for e in ea] + [0] * (na - len(ea))
                ib += [int(rowB[e]) for e in eb] + [0] * (nb_ - len(eb))
                il_a += [int(ld_all[e]) for e in ea] + [0] * (na - len(ea))
                il_b += [int(ld_all[e]) for e in eb] + [0] * (nb_ - len(eb))
                lv_a += [float(lin_all[e]) for e in ea] + [PAD_LDST] * (na - len(ea))
                lv_b += [float(lin_all[e]) for e in eb] + [PAD_LDST] * (nb_ - len(eb))
            if ia:
                idxA_cols.append(_wrap_idx(ia))
            if ib:
                idxB_cols.append(_wrap_idx(ib))
            il = il_a + il_b
            lv = lv_a + lv_b
            if il:
                idxL_cols.append(_wrap_idx(il))
                tg = len(lv) // P
                lcol = np.asarray(lv, np.float16).reshape(tg, P).T  # [128, tg]
                ldst_cols.append(np.repeat(lcol, 2, axis=1))        # [128, tg*2]

        in_maps.append({
            "xT2": xT2,
            "W1k": W1k, "W2k": W2k,
            "idxA": (np.concatenate(idxA_cols, axis=1) if idxA_cols
                     else np.zeros((P, 8), np.int16)),
            "idxB": (np.concatenate(idxB_cols, axis=1) if idxB_cols
                     else np.zeros((P, 8), np.int16)),
            "idxL": (np.concatenate(idxL_cols, axis=1) if idxL_cols
                     else np.zeros((P, 8), np.int16)),
            "ldstT": (np.concatenate(ldst_cols, axis=1) if ldst_cols
                      else np.zeros((P, 1), np.float16)),
            "iota": iota, "ident": ident,
        })
    plan["idxA_cols"] = in_maps[0]["idxA"].shape[1]
    plan["idxB_cols"] = in_maps[0]["idxB"].shape[1]
    plan["idxL_cols"] = in_maps[0]["idxL"].shape[1]
    plan["ldst_cols"] = in_maps[0]["ldstT"].shape[1]
    return in_maps, plan


def _build(plan):
    import os
    KLVL = int(os.environ.get("KLVL", "5"))
    KSIM = int(os.environ.get("KSIM", "0"))
    import concourse.bass as bass
    import concourse.bacc as bacc
    import concourse.mybir as mybir
    import concourse.tile as tile

    dt = mybir.dt
    ginfo = plan["ginfo"]

    nc = bacc.Bacc("TRN2", target_bir_lowering=False, debug=False,
                   num_devices=(1 if KSIM else N_CORES))
    xT2_ap = nc.dram_tensor("xT2", [P, 2 * NPAD], dt.float16, kind="ExternalInput").ap()
    W1k_ap = nc.dram_tensor("W1k", [P, 2 * 264], dt.float16, kind="ExternalInput").ap()
    W2k_ap = nc.dram_tensor("W2k", [P, 2 * 264], dt.float16, kind="ExternalInput").ap()
    idxA_ap = nc.dram_tensor("idxA", [P, plan["idxA_cols"]], dt.int16, kind="ExternalInput").ap()
    idxB_ap = nc.dram_tensor("idxB", [P, plan["idxB_cols"]], dt.int16, kind="ExternalInput").ap()
    idxL_ap = nc.dram_tensor("idxL", [P, plan["idxL_cols"]], dt.int16, kind="ExternalInput").ap()
    ldstT_ap = nc.dram_tensor("ldstT", [P, plan["ldst_cols"]], dt.float16, kind="ExternalInput").ap()
    iota_ap = nc.dram_tensor("iota", [P, P], dt.float16, kind="ExternalInput").ap()
    ident_ap = nc.dram_tensor("ident", [P, P], dt.float16, kind="ExternalInput").ap()
    out_ap = nc.dram_tensor("out", [NPAD, IN_DIM], dt.float16, kind="ExternalOutput").ap()

    with tile.TileContext(nc) as tc:
        with tc.tile_pool(name="const", bufs=1) as cpool, \
             tc.tile_pool(name="gemm", bufs=3) as gpool, \
             tc.tile_pool(name="edge", bufs=3) as epool, \
             tc.tile_pool(name="flush", bufs=2) as fpool, \
             tc.tile_pool(name="psum", bufs=2, space="PSUM") as pp, \
             tc.tile_pool(name="aggp", bufs=4, space="PSUM") as ap_pool, \
             tc.tile_pool(name="dram", bufs=1, space="DRAM") as dram:

            iota_t = cpool.tile([P, P], dt.float16)
            ident_t = cpool.tile([P, P], dt.float16)
            idxA_t = cpool.tile([P, plan["idxA_cols"]], dt.int16)
            idxB_t = cpool.tile([P, plan["idxB_cols"]], dt.int16)
            idxL_t = cpool.tile([P, plan["idxL_cols"]], dt.int16)
            ldst_t = cpool.tile([P, plan["ldst_cols"]], dt.float16)
            w1_t = cpool.tile([P, 2 * 264], dt.float16)
            w2_t = cpool.tile([P, 2 * 264], dt.float16)
            er_sb = cpool.tile([P, NB * 4], dt.float16)
            nc.sync.dma_start(iota_t[:], iota_ap[:])
            nc.sync.dma_start(ident_t[:], ident_ap[:])
            nc.sync.dma_start(idxA_t[:], idxA_ap[:])
            nc.sync.dma_start(idxB_t[:], idxB_ap[:])
            nc.sync.dma_start(idxL_t[:], idxL_ap[:])
            nc.sync.dma_start(ldst_t[:], ldstT_ap[:])
            nc.sync.dma_start(w1_t[:], W1k_ap[:])
            nc.sync.dma_start(w2_t[:], W2k_ap[:])

            tab_loc = dram.tile([NPAD, ES], dt.float16)
            _ashared = "Local" if KSIM else "Shared"
            tab1 = dram.tile([RTOT, ES], dt.float16, addr_space=_ashared)
            tab2 = dram.tile([RTOT, ES], dt.float16, addr_space=_ashared)
            er_pad = dram.tile([NPAD, ERES], dt.float16)
            h1T2 = dram.tile([P, 2 * NPAD], dt.float16)

            def gemm_block(layer, b):
                wk = w1_t if layer == 1 else w2_t
                hk = gpool.tile([P, 2 * P], dt.float16, name="hk", tag="hk")
                if layer == 1:
                    nc.sync.dma_start(hk[:], xT2_ap[:, b * 256:(b + 1) * 256])
                else:
                    nc.sync.dma_start(hk[:], h1T2[:, b * 256:(b + 1) * 256])
                ps = pp.tile([P, 264], dt.float32, space="PSUM", name="gemm_ps", tag="gemm_ps")
                for g in range(2):
                    nc.tensor.matmul(out=ps[:], lhsT=hk[:, g * P:(g + 1) * P],
                                     rhs=wk[:, g * 264:(g + 1) * 264],
                                     start=(g == 0), stop=(g == 1))
                sb = gpool.tile([P, 264], dt.float16, name="gemm_sb", tag="gemm_sb")
                nc.vector.tensor_copy(sb[:], ps[:])
                nc.sync.dma_start(tab_loc[b * P:(b + 1) * P, 0:CG], sb[:, 0:CG])
                nc.vector.tensor_copy(er_sb[:, b * 4:(b + 1) * 4], sb[:, 260:264])

            def er_flush():
                # er_sb [128, 49*4] -> er_pad rows (b*128+p), cols 0:4
                dst_ap = er_pad[:, 0:4].rearrange("(b p) e -> p b e", p=P)
                nc.sync.dma_start(dst_ap, er_sb[:].rearrange("p (b e) -> p b e", e=4))

            def edge_group(layer, gi, toff, aoff, boff):
                tA, tB = gi["tA"], gi["tB"]
                tgA, tgB = sum(tA), sum(tB)
                tg = tgA + tgB
                if tg == 0 or KLVL < 3:
                    return
                tab = tab1 if layer == 1 else tab2
                G = epool.tile([P, tg * ES], dt.float16, name="G", tag="G")
                if tgA:
                    nc.gpsimd.dma_gather(
                        out_ap=G[:, 0:tgA * ES].rearrange("p (t e) -> p t e", e=ES),
                        in_ap=tab[0:HALF, :], idxs_ap=idxA_t[:, 8 * aoff: 8 * (aoff + tgA)],
                        num_idxs=tgA * P, num_idxs_reg=tgA * P, elem_size=ES,
                        single_packet=False)
                if tgB:
                    nc.gpsimd.dma_gather(
                        out_ap=G[:, tgA * ES:tg * ES].rearrange("p (t e) -> p t e", e=ES),
                        in_ap=tab[HALF:RTOT, :], idxs_ap=idxB_t[:, 8 * boff: 8 * (boff + tgB)],
                        num_idxs=tgB * P, num_idxs_reg=tgB * P, elem_size=ES,
                        single_packet=False)
                ER = epool.tile([P, tg * ERES], dt.float16, name="ER", tag="ER")
                nc.gpsimd.dma_gather(
                    out_ap=ER[:].rearrange("p (t e) -> p t e", e=ERES),
                    in_ap=er_pad[:], idxs_ap=idxL_t[:, 8 * toff: 8 * (toff + tg)],
                    num_idxs=tg * P, num_idxs_reg=tg * P, elem_size=ERES,
                    single_packet=False)

                if KLVL < 4:
                    return
                g3 = G[:].rearrange("p (t c) -> p t c", c=ES)
                er3 = ER[:].rearrange("p (t c) -> p t c", c=ERES)
                E = epool.tile([P, tg * 4], dt.float16, name="E", tag="E")
                e3 = E[:].rearrange("p (t h) -> p t h", h=4)
                nc.vector.tensor_tensor(out=e3, in0=g3[:, :, 256:260],
                                        in1=er3[:, :, 0:4], op=mybir.AluOpType.add)
                L = epool.tile([P, tg * 4], dt.float16, name="L", tag="L")
                nc.vector.tensor_scalar_mul(L[:], E[:], NEG_SLOPE)
                nc.vector.tensor_tensor(out=L[:], in0=E[:], in1=L[:],
                                        op=mybir.AluOpType.max)
                XW = epool.tile([P, tg * 4], dt.float16, name="XW", tag="XW")
                nc.scalar.activation(XW[:], L[:], mybir.ActivationFunctionType.Exp)

                # fold alpha back into G in place: feat *= alpha, el slot := alpha
                nc.vector.tensor_copy(g3[:, :, 256:260],
                                      XW[:].rearrange("p (t h) -> p t h", h=4))
                # pair-expanded alpha so every operand's last AP dim is packed
                # (keeps the DVE 2x fp16 fast path; stride-0 last dims disable it)
                X2 = epool.tile([P, tg * 8], dt.float16, name="X2", tag="X2")
                nc.vector.tensor_copy(
                    X2[:].rearrange("p (q two) -> p q two", two=2),
                    XW[:].rearrange("p (q o) -> p q o", o=1).to_broadcast([P, tg * 4, 2]))
                x5 = X2[:].rearrange("p (t h two) -> p t h two", h=4, two=2)
                for h in range(4):
                    w4h = g3[:, :, h * 64:(h + 1) * 64] \
                        .rearrange("p t (a two) -> p t a two", two=2)
                    x4h = x5[:, :, h, :].rearrange("p t (o two) -> p t o two", o=1) \
                                        .to_broadcast([P, tg, 32, 2])
                    nc.vector.tensor_tensor(out=w4h, in0=w4h, in1=x4h,
                                            op=mybir.AluOpType.mult)

                MT = epool.tile([P, tg * P], dt.float16, name="MT", tag="MT")
                mt4 = MT[:].rearrange("p (t a two) -> p t a two", a=64, two=2)
                iota4 = iota_t[:].rearrange("p (o a two) -> p o a two", o=1, two=2) \
                                 .to_broadcast([P, tg, 64, 2])
                lds4 = ldst_t[:, 2 * toff:2 * (toff + tg)] \
                    .rearrange("p (t o two) -> p t o two", o=1, two=2) \
                    .to_broadcast([P, tg, 64, 2])
                nc.vector.tensor_tensor(out=mt4, in0=iota4, in1=lds4,
                                        op=mybir.AluOpType.is_equal)

                # per-block aggregation + flush
                for k, b in enumerate(gi["blks"]):
                    tiles = (list(range(sum(tA[:k]), sum(tA[:k]) + tA[k]))
                             + list(range(tgA + sum(tB[:k]), tgA + sum(tB[:k]) + tB[k])))
                    if not tiles:
                        continue
                    agg = ap_pool.tile([P, CG], dt.float32, space="PSUM", name="agg_ps", tag="agg_ps")
                    for j, ti in enumerate(tiles):
                        nc.tensor.matmul(out=agg[:], lhsT=MT[:, ti * P:(ti + 1) * P],
                                         rhs=G[:, ti * ES:ti * ES + CG],
                                         start=(j == 0), stop=(j == len(tiles) - 1))

                    dmx = fpool.tile([P, 4], dt.float32, name="dmx", tag="dmx")
                    nc.vector.tensor_scalar_max(dmx[:], agg[:, 256:260], 1e-30)
                    rec = fpool.tile([P, 4], dt.float32, name="rec", tag="rec")
                    nc.vector.reciprocal(rec[:], dmx[:])
                    ob = fpool.tile([P, 256], dt.float16, name="ob", tag="ob")
                    ob3 = ob[:].rearrange("p (h j) -> p h j", j=64)
                    rec3 = rec[:].rearrange("p (h o) -> p h o", o=1).to_broadcast([P, 4, 64])
                    nc.vector.tensor_tensor(out=ob3,
                                            in0=agg[:, 0:256].rearrange("p (h j) -> p h j", j=64),
                                            in1=rec3, op=mybir.AluOpType.mult)
                    nb_t = fpool.tile([P, 256], dt.float16, name="nb", tag="nb")
                    nc.vector.tensor_scalar_min(nb_t[:], ob[:], 0.0)
                    en = fpool.tile([P, 256], dt.float16, name="en", tag="en")
                    nc.scalar.activation(en[:], nb_t[:], mybir.ActivationFunctionType.Exp)
                    pb = fpool.tile([P, 256], dt.float16, name="pb", tag="pb")
                    nc.scalar.activation(pb[:], ob[:], mybir.ActivationFunctionType.Relu)
                    fb = fpool.tile([P, 256], dt.float16, name="fb", tag="fb")
                    nc.vector.tensor_tensor(out=fb[:], in0=en[:], in1=pb[:],
                                            op=mybir.AluOpType.add)
                    nc.vector.tensor_scalar_add(fb[:], fb[:], -1.0)

                    if KLVL < 5:
                        continue
                    if layer == 1:
                        tsb = fpool.tile([P, 2 * P], dt.float16, name="tsb", tag="tsb")
                        for g in range(2):
                            trp = pp.tile([P, P], dt.float16, space="PSUM", name="tr_ps", tag="tr_ps")
                            nc.tensor.transpose(out=trp[:], in_=fb[:, g * P:(g + 1) * P],
                                                identity=ident_t[:])
                            nc.vector.tensor_copy(tsb[:, g * P:(g + 1) * P], trp[:])
                        nc.sync.dma_start(h1T2[:, b * 256:(b + 1) * 256], tsb[:])
                    else:
                        nc.sync.dma_start(out_ap[b * P:(b + 1) * P, :], fb[:])

            def do_ag(layer):
                if KLVL >= 2 and not KSIM:
                    nc.gpsimd.collective_compute(
                        "AllGather", mybir.AluOpType.bypass,
                        replica_groups=[list(range(N_CORES))],
                        ins=[tab_loc.opt()],
                        outs=[(tab1 if layer == 1 else tab2).opt()])

            for b in range(NB):
                gemm_block(1, b)
            er_flush()
            do_ag(1)
            toff = aoff = boff = 0
            for gi in ginfo:
                edge_group(1, gi, toff, aoff, boff)
                toff += sum(gi["tA"]) + sum(gi["tB"])
                aoff += sum(gi["tA"])
                boff += sum(gi["tB"])
                # layer-2 GEMM for freshly flushed blocks overlaps edge phase
                if KLVL >= 5:
                    for b in gi["blks"]:
                        gemm_block(2, b)
            if KLVL >= 5:
                er_flush()
                do_ag(2)
                toff = aoff = boff = 0
                for gi in ginfo:
                    edge_group(2, gi, toff, aoff, boff)
                    toff += sum(gi["tA"]) + sum(gi["tB"])
                    aoff += sum(gi["tA"])
                    boff += sum(gi["tB"])
    nc.compile()
    return nc


def kernel(**inputs):
    from concourse.bass_utils import run_bass_kernel_spmd
    in_maps, plan = _prep(inputs["x"], inputs["src"], inputs["dst"],
                          inputs["W1"], inputs["al1"], inputs["ar1"],
                          inputs["W2"], inputs["al2"], inputs["ar2"])
    nc = _build(plan)
    res = run_bass_kernel_spmd(nc, in_maps, core_ids=list(range(N_CORES)),
                               trace=False)
    h = np.concatenate([res.results[c]["out"][:NPC] for c in range(N_CORES)],
                       axis=0).astype(np.float32)
    return tuple(h[:, i * HID:(i + 1) * HID] for i in range(HEADS))


# revision 12
# speedup vs baseline: 2.6454x; 2.6454x over previous
"""2-layer multi-head GAT on 8 Trainium2 NeuronCores.

Sharding: nodes partitioned across 8 cores by dst ownership (6250 real nodes
per core, padded to 6272 = 49x128). Edges live on their dst's core, sorted by
dst into 128-dst blocks. Per layer:
  1. per-core GEMM  feat|el|er = h @ [W | W@Al | W@Ar]  (fp16 in, fp32 psum)
     -> local table shard [6272, 384] fp16 rows (256 feat + 4 el), er kept
     in a separate local [6272, 128] fp16 table (256B gather rows)
  2. ONE AllGather publishes all shards -> [50176, 384] (core-major order;
     rows < 25088 indexed via table-half A, rest via half B so dma_gather
     int16 indices stay in range)
  3. per 2-block group: dma_gather of src rows (768B) + er rows (256B),
     attention e-chain (DVE/ACT), selection-matrix aggregation matmuls
     accumulated per dst-block in PSUM (fp16 operands, exact 0/1 lhsT)
  4. flush per block: divide by softmax denominators, ELU (fp16), transpose
     for the next GEMM / final output
"""
import sys
sys.path.insert(0, '/opt/trn_rl_repo')
import numpy as np

N_NODES = 50000
N_EDGES = 800000
IN_DIM = 256
HID = 64
HEADS = 4
NEG_SLOPE = 0.2
N_CORES = 8
NPC = N_NODES // N_CORES          # 6250 real nodes per core
P = 128
NB = 49                            # dst blocks per core
NPAD = NB * P                      # 6272 padded nodes per core
HALF = 4 * NPAD                    # 25088: first table half (cores 0-3)
RTOT = N_CORES * NPAD              # 50176 gathered table rows
ES = 384                           # table row elems fp16 (768B)
ERES = 128                         # er table row elems fp16 (256B)
CG = 260                           # feat + denom columns in agg matmul
GB = 2                             # dst blocks per gather group
PAD_LDST = 999.0


def _wrap_idx(idx_list):
    """[n] int -> [128, n//16] int16 wrapped-in-16 layout, replicated."""
    n = len(idx_list)
    assert n % 16 == 0
    arr = np.asarray(idx_list, np.int16).reshape(n // 16, 16)  # [s, q]
    w16 = arr.T                                                # [16, s]
    return np.tile(w16, (8, 1))                                # [128, s]


def _prep(x, src, dst, W1, al1, ar1, W2, al2, ar2, kdt=16):
    src = np.asarray(src).astype(np.int64)
    dst = np.asarray(dst).astype(np.int64)
    x = np.asarray(x, np.float32)

    own = (src // NPC).astype(np.int32)
    loc = (src % NPC).astype(np.int32)
    in_a = own < 4
    rowA = own * NPAD + loc                   # valid where in_a  (< 25088)
    rowB = (own - 4) * NPAD + loc             # valid where ~in_a (< 25088)

    core_of = (dst // NPC).astype(np.int32)
    ld_all = (dst % NPC).astype(np.int32)
    blk_all = ld_all // P
    lin_all = ld_all % P

    # per (core, block): lists of A-edges and B-edges
    eA = [[[] for _ in range(NB)] for _ in range(N_CORES)]
    eB = [[[] for _ in range(NB)] for _ in range(N_CORES)]
    order = np.lexsort((src, dst))
    for e in order:
        c = core_of[e]
        b = blk_all[e]
        (eA if in_a[e] else eB)[c][b].append(e)

    T_A = [max(len(eA[c][b]) for c in range(N_CORES)) for b in range(NB)]
    T_B = [max(len(eB[c][b]) for c in range(N_CORES)) for b in range(NB)]
    T_A = [-(-n // P) for n in T_A]
    T_B = [-(-n // P) for n in T_B]

    # groups of GB consecutive blocks
    groups = [list(range(g, min(g + GB, NB))) for g in range(0, NB, GB)]
    # per group: tile layout [A(b0) A(b1) ... B(b0) B(b1) ...]
    ginfo = []
    for blks in groups:
        ginfo.append({
            "blks": blks,
            "tA": [T_A[b] for b in blks],
            "tB": [T_B[b] for b in blks],
        })
    plan = {"ginfo": ginfo, "T_A": T_A, "T_B": T_B}

    # attention projection: [256, 4] per layer with per-head blocks
    def aext(al, ar):
        Al = np.zeros((IN_DIM, HEADS), np.float64)
        Ar = np.zeros((IN_DIM, HEADS), np.float64)
        for h in range(HEADS):
            Al[h * HID:(h + 1) * HID, h] = np.asarray(al, np.float64)[h]
            Ar[h * HID:(h + 1) * HID, h] = np.asarray(ar, np.float64)[h]
        return Al, Ar

    Al1, Ar1 = aext(al1, ar1)
    Al2, Ar2 = aext(al2, ar2)

    def wext(W, Al, Ar):
        W = np.asarray(W, np.float64)
        m = np.concatenate([W, W @ Al, W @ Ar], axis=1)  # [256, 264]
        out = np.zeros((P, 2 * 264), np.float16)
        for g in range(2):
            out[:, g * 264:(g + 1) * 264] = m[g * P:(g + 1) * P].astype(np.float16)
        return out

    W1k = wext(W1, Al1, Ar1)
    W2k = wext(W2, Al2, Ar2)
    iota = np.tile(np.arange(P, dtype=np.float16), (P, 1))
    ident = np.eye(P, dtype=np.float16)

    in_maps = []
    for c in range(N_CORES):
        xl = np.zeros((NPAD, IN_DIM), np.float32)
        xl[:NPC] = x[c * NPC:(c + 1) * NPC]
        # block-interleaved transpose: xT2[p, b*256 + g*128 + n] = xl[b*128+n, g*128+p]
        xT2 = np.ascontiguousarray(
            xl.reshape(NB, P, 2, P).transpose(3, 0, 2, 1).reshape(P, 2 * NPAD)
        ).astype(np.float16)

        idxA_cols = []
        idxB_cols = []
        idxL_cols = []
        ldst_cols = []
        for gi in ginfo:
            ia, ib, il_a, il_b, lv_a, lv_b = [], [], [], [], [], []
            for k, b in enumerate(gi["blks"]):
                ea, eb = eA[c][b], eB[c][b]
                na, nb_ = gi["tA"][k] * P, gi["tB"][k] * P
                ia += [int(rowA[e]) for e in ea] + [0] * (na - len(ea))
                ib += [int(rowB[e]) for e in eb] + [0] * (nb_ - len(eb))
                il_a += [int(ld_all[e]) for e in ea] + [0] * (na - len(ea))
                il_b += [int(ld_all[e]) for e in eb] + [0] * (nb_ - len(eb))
                lv_a += [float(lin_all[e]) for e in ea] + [PAD_LDST] * (na - len(ea))
                lv_b += [float(lin_all[e]) for e in eb] + [PAD_LDST] * (nb_ - len(eb))
            if ia:
                idxA_cols.append(_wrap_idx(ia))
            if ib:
                idxB_cols.append(_wrap_idx(ib))
            il = il_a + il_b
            lv = lv_a + lv_b
            if il:
                idxL_cols.append(_wrap_idx(il))
                tg = len(lv) // P
                lcol = np.asarray(lv, np.float16).reshape(tg, P).T  # [128, tg]
                ldst_cols.append(np.repeat(lcol, 2, axis=1))        # [128, tg*2]

        in_maps.append({
            "xT2": xT2,
            "W1k": W1k, "W2k": W2k,
            "idxA": (np.concatenate(idxA_cols, axis=1) if idxA_cols
                     else np.zeros((P, 8), np.int16)),
            "idxB": (np.concatenate(idxB_cols, axis=1) if idxB_cols
                     else np.zeros((P, 8), np.int16)),
            "idxL": (np.concatenate(idxL_cols, axis=1) if idxL_cols
                     else np.zeros((P, 8), np.int16)),
            "ldstT": (np.concatenate(ldst_cols, axis=1) if ldst_cols
                      else np.zeros((P, 1), np.float16)),
            "iota": iota, "ident": ident,
        })
    plan["idxA_cols"] = in_maps[0]["idxA"].shape[1]
    plan["idxB_cols"] = in_maps[0]["idxB"].shape[1]
    plan["idxL_cols"] = in_maps[0]["idxL"].shape[1]
    plan["ldst_cols"] = in_maps[0]["ldstT"].shape[1]
    return in_maps, plan


def _build(plan):
    import os
    KLVL = int(os.environ.get("KLVL", "5"))
    KSIM = int(os.environ.get("KSIM", "0"))
    import concourse.bass as bass
    import concourse.bacc as bacc
    import concourse.mybir as mybir
    import concourse.tile as tile

    dt = mybir.dt
    ginfo = plan["ginfo"]

    KQN = int(os.environ.get("KQN", "1"))
    nc = bacc.Bacc("TRN2", target_bir_lowering=False, debug=False,
                   num_devices=(1 if KSIM else N_CORES),
                   num_swdge_queues=KQN)
    xT2_ap = nc.dram_tensor("xT2", [P, 2 * NPAD], dt.float16, kind="ExternalInput").ap()
    W1k_ap = nc.dram_tensor("W1k", [P, 2 * 264], dt.float16, kind="ExternalInput").ap()
    W2k_ap = nc.dram_tensor("W2k", [P, 2 * 264], dt.float16, kind="ExternalInput").ap()
    idxA_ap = nc.dram_tensor("idxA", [P, plan["idxA_cols"]], dt.int16, kind="ExternalInput").ap()
    idxB_ap = nc.dram_tensor("idxB", [P, plan["idxB_cols"]], dt.int16, kind="ExternalInput").ap()
    idxL_ap = nc.dram_tensor("idxL", [P, plan["idxL_cols"]], dt.int16, kind="ExternalInput").ap()
    ldstT_ap = nc.dram_tensor("ldstT", [P, plan["ldst_cols"]], dt.float16, kind="ExternalInput").ap()
    iota_ap = nc.dram_tensor("iota", [P, P], dt.float16, kind="ExternalInput").ap()
    ident_ap = nc.dram_tensor("ident", [P, P], dt.float16, kind="ExternalInput").ap()
    out_ap = nc.dram_tensor("out", [NPAD, IN_DIM], dt.float16, kind="ExternalOutput").ap()

    with tile.TileContext(nc) as tc:
        with tc.tile_pool(name="const", bufs=1) as cpool, \
             tc.tile_pool(name="gemm", bufs=3) as gpool, \
             tc.tile_pool(name="edge", bufs=3) as epool, \
             tc.tile_pool(name="flush", bufs=2) as fpool, \
             tc.tile_pool(name="psum", bufs=2, space="PSUM") as pp, \
             tc.tile_pool(name="aggp", bufs=4, space="PSUM") as ap_pool, \
             tc.tile_pool(name="dram", bufs=1, space="DRAM") as dram:

            iota_t = cpool.tile([P, P], dt.float16)
            ident_t = cpool.tile([P, P], dt.float16)
            idxA_t = cpool.tile([P, plan["idxA_cols"]], dt.int16)
            idxB_t = cpool.tile([P, plan["idxB_cols"]], dt.int16)
            idxL_t = cpool.tile([P, plan["idxL_cols"]], dt.int16)
            ldst_t = cpool.tile([P, plan["ldst_cols"]], dt.float16)
            w1_t = cpool.tile([P, 2 * 264], dt.float16)
            w2_t = cpool.tile([P, 2 * 264], dt.float16)
            er_sb = cpool.tile([P, NB * 4], dt.float16)
            nc.sync.dma_start(iota_t[:], iota_ap[:])
            nc.sync.dma_start(ident_t[:], ident_ap[:])
            nc.sync.dma_start(idxA_t[:], idxA_ap[:])
            nc.sync.dma_start(idxB_t[:], idxB_ap[:])
            nc.sync.dma_start(idxL_t[:], idxL_ap[:])
            nc.sync.dma_start(ldst_t[:], ldstT_ap[:])
            nc.sync.dma_start(w1_t[:], W1k_ap[:])
            nc.sync.dma_start(w2_t[:], W2k_ap[:])

            tab_loc = dram.tile([NPAD, ES], dt.float16)
            _ashared = "Local" if KSIM else "Shared"
            tab1 = dram.tile([RTOT, ES], dt.float16, addr_space=_ashared)
            tab2 = dram.tile([RTOT, ES], dt.float16, addr_space=_ashared)
            er_pad = dram.tile([NPAD, ERES], dt.float16)
            h1T2 = dram.tile([P, 2 * NPAD], dt.float16)

            def gemm_block(layer, b):
                wk = w1_t if layer == 1 else w2_t
                hk = gpool.tile([P, 2 * P], dt.float16, name="hk", tag="hk")
                if layer == 1:
                    nc.sync.dma_start(hk[:], xT2_ap[:, b * 256:(b + 1) * 256])
                else:
                    nc.sync.dma_start(hk[:], h1T2[:, b * 256:(b + 1) * 256])
                ps = pp.tile([P, 264], dt.float32, space="PSUM", name="gemm_ps", tag="gemm_ps")
                for g in range(2):
                    nc.tensor.matmul(out=ps[:], lhsT=hk[:, g * P:(g + 1) * P],
                                     rhs=wk[:, g * 264:(g + 1) * 264],
                                     start=(g == 0), stop=(g == 1))
                sb = gpool.tile([P, 264], dt.float16, name="gemm_sb", tag="gemm_sb")
                nc.vector.tensor_copy(sb[:], ps[:])
                nc.sync.dma_start(tab_loc[b * P:(b + 1) * P, 0:CG], sb[:, 0:CG])
                nc.vector.tensor_copy(er_sb[:, b * 4:(b + 1) * 4], sb[:, 260:264])

            def er_flush():
                # er_sb [128, 49*4] -> er_pad rows (b*128+p), cols 0:4
                dst_ap = er_pad[:, 0:4].rearrange("(b p) e -> p b e", p=P)
                nc.sync.dma_start(dst_ap, er_sb[:].rearrange("p (b e) -> p b e", e=4))

            def edge_group(layer, gi, toff, aoff, boff):
                tA, tB = gi["tA"], gi["tB"]
                tgA, tgB = sum(tA), sum(tB)
                tg = tgA + tgB
                if tg == 0 or KLVL < 3:
                    return
                tab = tab1 if layer == 1 else tab2
                G = epool.tile([P, tg * ES], dt.float16, name="G", tag="G")
                if tgA:
                    nc.gpsimd.dma_gather(
                        out_ap=G[:, 0:tgA * ES].rearrange("p (t e) -> p t e", e=ES),
                        in_ap=tab[0:HALF, :], idxs_ap=idxA_t[:, 8 * aoff: 8 * (aoff + tgA)],
                        num_idxs=tgA * P, num_idxs_reg=tgA * P, elem_size=ES,
                        single_packet=False)
                if tgB:
                    nc.gpsimd.dma_gather(
                        out_ap=G[:, tgA * ES:tg * ES].rearrange("p (t e) -> p t e", e=ES),
                        in_ap=tab[HALF:RTOT, :], idxs_ap=idxB_t[:, 8 * boff: 8 * (boff + tgB)],
                        num_idxs=tgB * P, num_idxs_reg=tgB * P, elem_size=ES,
                        single_packet=False, queue_num=(1 if KQN > 1 else 0))
                ER = epool.tile([P, tg * ERES], dt.float16, name="ER", tag="ER")
                nc.gpsimd.dma_gather(
                    out_ap=ER[:].rearrange("p (t e) -> p t e", e=ERES),
                    in_ap=er_pad[:], idxs_ap=idxL_t[:, 8 * toff: 8 * (toff + tg)],
                    num_idxs=tg * P, num_idxs_reg=tg * P, elem_size=ERES,
                    single_packet=False, queue_num=(2 if KQN > 2 else 0))

                if KLVL < 4:
                    return
                g3 = G[:].rearrange("p (t c) -> p t c", c=ES)
                er3 = ER[:].rearrange("p (t c) -> p t c", c=ERES)
                E = epool.tile([P, tg * 4], dt.float16, name="E", tag="E")
                e3 = E[:].rearrange("p (t h) -> p t h", h=4)
                nc.vector.tensor_tensor(out=e3, in0=g3[:, :, 256:260],
                                        in1=er3[:, :, 0:4], op=mybir.AluOpType.add)
                L = epool.tile([P, tg * 4], dt.float16, name="L", tag="L")
                nc.vector.tensor_scalar_mul(L[:], E[:], NEG_SLOPE)
                nc.vector.tensor_tensor(out=L[:], in0=E[:], in1=L[:],
                                        op=mybir.AluOpType.max)
                XW = epool.tile([P, tg * 4], dt.float16, name="XW", tag="XW")
                nc.scalar.activation(XW[:], L[:], mybir.ActivationFunctionType.Exp)

                # fold alpha back into G in place: feat *= alpha, el slot := alpha
                nc.vector.tensor_copy(g3[:, :, 256:260],
                                      XW[:].rearrange("p (t h) -> p t h", h=4))
                # pair-expanded alpha so every operand's last AP dim is packed
                # (keeps the DVE 2x fp16 fast path; stride-0 last dims disable it)
                X2 = epool.tile([P, tg * 8], dt.float16, name="X2", tag="X2")
                nc.vector.tensor_copy(
                    X2[:].rearrange("p (q two) -> p q two", two=2),
                    XW[:].rearrange("p (q o) -> p q o", o=1).to_broadcast([P, tg * 4, 2]))
                x5 = X2[:].rearrange("p (t h two) -> p t h two", h=4, two=2)
                for h in range(4):
                    w4h = g3[:, :, h * 64:(h + 1) * 64] \
                        .rearrange("p t (a two) -> p t a two", two=2)
                    x4h = x5[:, :, h, :].rearrange("p t (o two) -> p t o two", o=1) \
                                        .to_broadcast([P, tg, 32, 2])
                    nc.vector.tensor_tensor(out=w4h, in0=w4h, in1=x4h,
                                            op=mybir.AluOpType.mult)

                MT = epool.tile([P, tg * P], dt.float16, name="MT", tag="MT")
                mt4 = MT[:].rearrange("p (t a two) -> p t a two", a=64, two=2)
                iota4 = iota_t[:].rearrange("p (o a two) -> p o a two", o=1, two=2) \
                                 .to_broadcast([P, tg, 64, 2])
                lds4 = ldst_t[:, 2 * toff:2 * (toff + tg)] \
                    .rearrange("p (t o two) -> p t o two", o=1, two=2) \
                    .to_broadcast([P, tg, 64, 2])
                nc.vector.tensor_tensor(out=mt4, in0=iota4, in1=lds4,
                                        op=mybir.AluOpType.is_equal)

                # per-block aggregation + flush
                for k, b in enumerate(gi["blks"]):
                    tiles = (list(range(sum(tA[:k]), sum(tA[:k]) + tA[k]))
                             + list(range(tgA + sum(tB[:k]), tgA + sum(tB[:k]) + tB[k])))
                    if not tiles:
                        continue
                    agg = ap_pool.tile([P, CG], dt.float32, space="PSUM", name="agg_ps", tag="agg_ps")
                    for j, ti in enumerate(tiles):
                        nc.tensor.matmul(out=agg[:], lhsT=MT[:, ti * P:(ti + 1) * P],
                                         rhs=G[:, ti * ES:ti * ES + CG],
                                         start=(j == 0), stop=(j == len(tiles) - 1))

                    dmx = fpool.tile([P, 4], dt.float32, name="dmx", tag="dmx")
                    nc.vector.tensor_scalar_max(dmx[:], agg[:, 256:260], 1e-30)
                    rec = fpool.tile([P, 4], dt.float32, name="rec", tag="rec")
                    nc.vector.reciprocal(rec[:], dmx[:])
                    ob = fpool.tile([P, 256], dt.float16, name="ob", tag="ob")
                    ob3 = ob[:].rearrange("p (h j) -> p h j", j=64)
                    rec3 = rec[:].rearrange("p (h o) -> p h o", o=1).to_broadcast([P, 4, 64])
                    nc.vector.tensor_tensor(out=ob3,
                                            in0=agg[:, 0:256].rearrange("p (h j) -> p h j", j=64),
                                            in1=rec3, op=mybir.AluOpType.mult)
                    nb_t = fpool.tile([P, 256], dt.float16, name="nb", tag="nb")
                    nc.vector.tensor_scalar_min(nb_t[:], ob[:], 0.0)
                    en = fpool.tile([P, 256], dt.float16, name="en", tag="en")
                    nc.scalar.activation(en[:], nb_t[:], mybir.ActivationFunctionType.Exp)
                    pb = fpool.tile([P, 256], dt.float16, name="pb", tag="pb")
                    nc.scalar.activation(pb[:], ob[:], mybir.ActivationFunctionType.Relu)
                    fb = fpool.tile([P, 256], dt.float16, name="fb", tag="fb")
                    nc.vector.tensor_tensor(out=fb[:], in0=en[:], in1=pb[:],
                                            op=mybir.AluOpType.add)
                    nc.vector.tensor_scalar_add(fb[:], fb[:], -1.0)

                    if KLVL < 5:
                        continue
                    if layer == 1:
                        tsb = fpool.tile([P, 2 * P], dt.float16, name="tsb", tag="tsb")
                        for g in range(2):
                            trp = pp.tile([P, P], dt.float16, space="PSUM", name="tr_ps", tag="tr_ps")
                            nc.tensor.transpose(out=trp[:], in_=fb[:, g * P:(g + 1) * P],
                                                identity=ident_t[:])
                            nc.vector.tensor_copy(tsb[:, g * P:(g + 1) * P], trp[:])
                        nc.sync.dma_start(h1T2[:, b * 256:(b + 1) * 256], tsb[:])
                    else:
                        nc.sync.dma_start(out_ap[b * P:(b + 1) * P, :], fb[:])

            def do_ag(layer):
                if KLVL >= 2 and not KSIM:
                    nc.gpsimd.collective_compute(
                        "AllGather", mybir.AluOpType.bypass,
                        replica_groups=[list(range(N_CORES))],
                        ins=[tab_loc.opt()],
                        outs=[(tab1 if layer == 1 else tab2).opt()])

            for b in range(NB):
                gemm_block(1, b)
            er_flush()
            do_ag(1)
            toff = aoff = boff = 0
            for gi in ginfo:
                edge_group(1, gi, toff, aoff, boff)
                toff += sum(gi["tA"]) + sum(gi["tB"])
                aoff += sum(gi["tA"])
                boff += sum(gi["tB"])
                # layer-2 GEMM for freshly flushed blocks overlaps edge phase
                if KLVL >= 5:
                    for b in gi["blks"]:
                        gemm_block(2, b)
            if KLVL >= 5:
                er_flush()
                do_ag(2)
                toff = aoff = boff = 0
                for gi in ginfo:
                    edge_group(2, gi, toff, aoff, boff)
                    toff += sum(gi["tA"]) + sum(gi["tB"])
                    aoff += sum(gi["tA"])
                    boff += sum(gi["tB"])
    nc.compile()
    return nc


def kernel(**inputs):
    from concourse.bass_utils import run_bass_kernel_spmd
    in_maps, plan = _prep(inputs["x"], inputs["src"], inputs["dst"],
                          inputs["W1"], inputs["al1"], inputs["ar1"],
                          inputs["W2"], inputs["al2"], inputs["ar2"])
    nc = _build(plan)
    res = run_bass_kernel_spmd(nc, in_maps, core_ids=list(range(N_CORES)),
                               trace=False)
    h = np.concatenate([res.results[c]["out"][:NPC] for c in range(N_CORES)],
                       axis=0).astype(np.float32)
    return tuple(h[:, i * HID:(i + 1) * HID] for i in range(HEADS))
